# revision 1
# baseline (speedup 1.0000x reference)
"""Trainium2 Bass kernel for nn_BrainGeneratorModel (bias-field corrupt + per-sample
separable Gaussian blur + label LUT remap), 8-core data/spatial parallel.

Sharding: 8 cores = (sample b in 0..3) x (H-half in 0..1). Each core processes a
[D=192, H=96(+12 halo), W=192] subvolume of one sample plus its label slice.

Per-core pipeline (all blurs as PE matmuls against host-built banded matrices):
  A) stream d-batches: bias matmul (K=4) -> exp (ACT) -> x*expb (GPSIMD, bf16 out)
     -> H-blur matmul -> PE transpose (w onto partitions) -> W-blur matmul
     -> Y2 scratch in DRAM (f32)
  B) stream h-batches from Y2: PE transpose (d onto partitions) -> D-blur matmul
     -> img out (PSUM->DRAM direct)
  L) labels: 32-entry LUT as 32 fused is_equal*T[k] tensor_scalar + accumulate
     scalar_tensor_tensor passes on DVE (int16).
"""

import sys

for _p in ("/opt/trn_rl_repo",):
    if _p not in sys.path:
        sys.path.insert(0, _p)

import numpy as np
import ml_dtypes

import concourse.bass as bass
import concourse.mybir as mybir
import concourse.bacc as bacc
import concourse.tile as tile
from concourse.bass_utils import run_bass_kernel_spmd

F32 = mybir.dt.float32
BF16 = mybir.dt.bfloat16
I16 = mybir.dt.int16
I32 = mybir.dt.int32

B, C, D, H, W = 4, 1, 192, 192, 192
SMALL = 4
BIAS_STD = 0.7
MAX_SIGMA = 3.0
TRUNCATE = 4.0
K = 2 * int(TRUNCATE * MAX_SIGMA) + 1  # 25
P = K // 2  # 12
N_LABELS = 32
TABLE = 128

HC = 96            # interior H rows per core
HS = 120           # slab rows = HC + 2*P
DB = 8             # d-batch size (stage A)
NB_A = D // DB     # 24 batches
HB = 8             # h-batch size (stage B)
NB_B = HC // HB    # 12 batches
FA = DB * W        # 1536 stage-A free size
FLAB = D * HC * W // 128  # 27648 label cols per partition

_CACHE = {}


def _lin_weights(n_in, n_out):
    pos = np.linspace(0.0, n_in - 1.0, n_out, dtype=np.float64)
    i0 = np.clip(np.floor(pos).astype(np.int64), 0, n_in - 2)
    f = pos - i0
    Wm = np.zeros((n_out, n_in), np.float64)
    r = np.arange(n_out)
    np.add.at(Wm, (r, i0), 1.0 - f)
    np.add.at(Wm, (r, i0 + 1), f)
    return Wm


def _gauss_kernels(sigma3):
    """sigma3 [3] -> [3, K] kernels exactly as reference."""
    ar = np.arange(K, dtype=np.float64) - K // 2
    out = np.zeros((3, K), np.float64)
    for i, sg in enumerate(sigma3):
        s = max(float(sg), 1e-3)
        g = np.exp(-0.5 * ar * ar / (s * s))
        g = g / g.sum()
        if float(sg) >= 0.01:
            out[i] = g
        else:
            out[i, K // 2] = 1.0
    return out


def _edge_folded_toeplitz(g, n):
    """[n, n] matrix M with out[j] = sum_i M[i, j] * x[i], replicate padding."""
    M = np.zeros((n, n), np.float64)
    for j in range(n):
        for t in range(K):
            src = min(max(j + t - P, 0), n - 1)
            M[src, j] += g[t]
    return M


def _slab_toeplitz(g):
    """[HS, HC]: slab rows (pre-clipped by host) -> interior outputs."""
    M = np.zeros((HS, HC), np.float64)
    for j in range(HC):
        for t in range(K):
            M[j + t, j] += g[t]
    return M


def _build_program():
    nc = bacc.Bacc("TRN2", target_bir_lowering=False, debug=False)

    # ---- external inputs (per core) ----
    xs_h = nc.dram_tensor("xs", [D, HS, W], F32, kind="ExternalInput")
    c_h = nc.dram_tensor("cydw", [4, D * W], BF16, kind="ExternalInput")
    wht_h = nc.dram_tensor("wht", [4, HS], BF16, kind="ExternalInput")
    gh_h = nc.dram_tensor("gh", [HS, HC], BF16, kind="ExternalInput")
    gw_h = nc.dram_tensor("gw", [W, W], BF16, kind="ExternalInput")
    gd_h = nc.dram_tensor("gd", [D, D], BF16, kind="ExternalInput")
    lab_h = nc.dram_tensor("lab", [128, FLAB], I16, kind="ExternalInput")
    tab_h = nc.dram_tensor("tab", [128, N_LABELS], F32, kind="ExternalInput")
    id_h = nc.dram_tensor("idm", [128, 128], BF16, kind="ExternalInput")
    idf_h = nc.dram_tensor("idmf", [128, 128], F32, kind="ExternalInput")

    # ---- external outputs ----
    img_h = nc.dram_tensor("img", [D, HC, W], F32, kind="ExternalOutput")
    labo_h = nc.dram_tensor("labo", [128, FLAB], I16, kind="ExternalOutput")

    with tile.TileContext(nc) as tc:
        with (
            tc.tile_pool(name="consts", bufs=1) as cst,
            tc.tile_pool(name="sxp", bufs=2) as sxp,
            tc.tile_pool(name="cbp", bufs=2) as cbp,
            tc.tile_pool(name="ebp", bufs=2) as ebp,
            tc.tile_pool(name="xbp", bufs=2) as xbp,
            tc.tile_pool(name="xhp", bufs=2) as xhp,
            tc.tile_pool(name="zwp", bufs=2) as zwp,
            tc.tile_pool(name="ybp", bufs=2) as ybp,
            tc.tile_pool(name="zdp", bufs=2) as zdp,
            tc.tile_pool(name="zvp", bufs=2) as zvp,
            tc.tile_pool(name="zip", bufs=2) as zip_,
            tc.tile_pool(name="labp", bufs=1) as labp,
            tc.tile_pool(name="ps", bufs=8, space="PSUM") as psp,
            tc.tile_pool(name="dram", bufs=1, space="DRAM") as drp,
        ):
            # ---- constants to SBUF ----
            ght = cst.tile([HS, HC], BF16)
            nc.sync.dma_start(ght[:], gh_h.ap())
            gwa = cst.tile([128, W], BF16)
            nc.sync.dma_start(gwa[:], gw_h.ap()[0:128, :])
            gwb = cst.tile([64, W], BF16)
            nc.sync.dma_start(gwb[:], gw_h.ap()[128:192, :])
            gda = cst.tile([128, D], BF16)
            nc.sync.dma_start(gda[:], gd_h.ap()[0:128, :])
            gdb = cst.tile([64, D], BF16)
            nc.sync.dma_start(gdb[:], gd_h.ap()[128:192, :])
            whtt = cst.tile([4, HS], BF16)
            nc.sync.dma_start(whtt[:], wht_h.ap())
            idt = cst.tile([128, 128], BF16)
            nc.sync.dma_start(idt[:], id_h.ap())
            idft = cst.tile([128, 128], F32)
            nc.sync.dma_start(idft[:], idf_h.ap())
            tabt = cst.tile([128, N_LABELS], F32)
            nc.sync.dma_start(tabt[:], tab_h.ap())

            y2 = drp.tile([W, HC * D], F32)  # scratch [w', h', d]

            # ================= labels (DVE only, 2 chunks) =================
            FL2 = FLAB // 2
            for cc in range(2):
                lsl = slice(cc * FL2, (cc + 1) * FL2)
                lt = labp.tile([128, FL2], I16, tag="lt")
                nc.sync.dma_start(lt[:], lab_h.ap()[:, lsl])
                acc = labp.tile([128, FL2], I16, tag="acc")
                ek = labp.tile([128, FL2], I16, tag="ek")
                nc.vector.tensor_scalar(
                    acc[:], lt[:], 0, tabt[:, 0:1],
                    mybir.AluOpType.is_equal, mybir.AluOpType.mult)
                for k in range(1, N_LABELS):
                    nc.vector.tensor_scalar(
                        ek[:], lt[:], k, tabt[:, k:k + 1],
                        mybir.AluOpType.is_equal, mybir.AluOpType.mult)
                    nc.vector.scalar_tensor_tensor(
                        acc[:], ek[:], 1, acc[:],
                        mybir.AluOpType.mult, mybir.AluOpType.add)
                nc.sync.dma_start(labo_h.ap()[:, lsl], acc[:])

            # ================= stage A =================
            for ib in range(NB_A):
                d0 = ib * DB
                sx = sxp.tile([HS, FA], F32)
                # src: partition h (stride W), free (d: stride HS*W, w: 1)
                nc.sync.dma_start(
                    sx[:],
                    bass.AP(xs_h, d0 * HS * W, [[W, HS], [HS * W, DB], [1, W]]),
                )
                cb = cbp.tile([4, FA], BF16)
                nc.sync.dma_start(cb[:], c_h.ap()[:, d0 * W:(d0 + DB) * W])

                xb = xbp.tile([HS, FA], BF16)
                for q in range(FA // 512):
                    sl = slice(q * 512, (q + 1) * 512)
                    psb = psp.tile([HS, 512], F32, tag="ps")
                    nc.tensor.matmul(psb[:], whtt[:], cb[:, sl], start=True, stop=True)
                    eb = ebp.tile([HS, 512], F32)
                    nc.scalar.activation(eb[:], psb[:], mybir.ActivationFunctionType.Exp)
                    nc.gpsimd.tensor_tensor(xb[:, sl], sx[:, sl], eb[:], mybir.AluOpType.mult)

                xh = xhp.tile([HC, FA], BF16)
                for q in range(FA // 512):
                    sl = slice(q * 512, (q + 1) * 512)
                    psh = psp.tile([HC, 512], F32, tag="ps")
                    nc.tensor.matmul(psh[:], ght[:], xb[:, sl], start=True, stop=True)
                    nc.scalar.copy(xh[:, sl], psh[:])

                # T1: w onto partitions. zw free layout: (h', dl): idx = hp*DB + dl
                zwa = zwp.tile([128, HC * DB], BF16, tag="zwa")
                zwb = zwp.tile([64, HC * DB], BF16, tag="zwb")
                for g in range(DB // 4):
                    pta = psp.tile([128, 4 * HC], BF16, tag="ps")
                    ptb = psp.tile([64, 4 * HC], BF16, tag="ps")
                    for t in range(4):
                        dl = g * 4 + t
                        nc.tensor.transpose(
                            pta[:, t * HC:(t + 1) * HC],
                            xh[:, dl * W: dl * W + 128], idt[0:HC, 0:HC])
                        nc.tensor.transpose(
                            ptb[:, t * HC:(t + 1) * HC],
                            xh[:, dl * W + 128: dl * W + 192], idt[0:HC, 0:HC])
                    # copy psum->zw with (t outer, h' inner) -> (dl, h'*DB+dl)
                    nc.scalar.copy(
                        zwa[:].rearrange("p (h d) -> p d h", d=DB)[:, g * 4:(g + 1) * 4, :],
                        pta[:].rearrange("p (t h) -> p t h", t=4),
                    )
                    nc.scalar.copy(
                        zwb[:].rearrange("p (h d) -> p d h", d=DB)[:, g * 4:(g + 1) * 4, :],
                        ptb[:].rearrange("p (t h) -> p t h", t=4),
                    )

                # W-blur -> SBUF staging -> y2 [w', h', d]
                nfree = HC * DB  # 768
                for m in range(2):
                    msl = slice(m * 96, (m + 1) * 96)
                    zv = zvp.tile([96, nfree], F32, tag="zv")
                    for q in range(nfree // 384):
                        sl = slice(q * 384, (q + 1) * 384)  # 48 h' x DB dl
                        psw = psp.tile([96, 384], F32, tag="ps")
                        nc.tensor.matmul(psw[:], gwa[:, msl], zwa[:, sl], start=True, stop=False)
                        nc.tensor.matmul(psw[:], gwb[:, msl], zwb[:, sl], start=False, stop=True)
                        nc.scalar.copy(zv[:, sl], psw[:])
                    # zv free = (h' 96, dl 8); y2 free = h'*D + d
                    nc.sync.dma_start(
                        bass.AP(y2.tensor,
                                y2[:].offset + m * 96 * HC * D + d0,
                                [[HC * D, 96], [D, HC], [1, DB]]),
                        zv[:],
                    )

            # ================= stage B =================
            for jb in range(NB_B):
                h0 = jb * HB
                yba = ybp.tile([96, HB * D], F32, tag="yba")
                ybb = ybp.tile([96, HB * D], F32, tag="ybb")
                nc.sync.dma_start(yba[:], bass.AP(y2.tensor, y2[:].offset + h0 * D,
                                                  [[HC * D, 96], [1, HB * D]]))
                nc.sync.dma_start(ybb[:], bass.AP(y2.tensor,
                                                  y2[:].offset + 96 * HC * D + h0 * D,
                                                  [[HC * D, 96], [1, HB * D]]))
                zda = zdp.tile([128, HB * W], BF16, tag="zda")
                zdb = zdp.tile([64, HB * W], BF16, tag="zdb")
                for g in range(HB // 2):
                    pta = psp.tile([128, 384], F32, tag="ps")
                    ptb = psp.tile([64, 384], F32, tag="ps")
                    for t in range(2):
                        hl = g * 2 + t
                        nc.tensor.transpose(
                            pta[:, t * 192 + 0: t * 192 + 96],
                            yba[:, hl * D + 0: hl * D + 128], idft[0:96, 0:96])
                        nc.tensor.transpose(
                            pta[:, t * 192 + 96: t * 192 + 192],
                            ybb[:, hl * D + 0: hl * D + 128], idft[0:96, 0:96])
                        nc.tensor.transpose(
                            ptb[:, t * 192 + 0: t * 192 + 96],
                            yba[:, hl * D + 128: hl * D + 192], idft[0:96, 0:96])
                        nc.tensor.transpose(
                            ptb[:, t * 192 + 96: t * 192 + 192],
                            ybb[:, hl * D + 128: hl * D + 192], idft[0:96, 0:96])
                    nc.scalar.copy(zda[:, g * 384:(g + 1) * 384], pta[:])
                    nc.scalar.copy(zdb[:, g * 384:(g + 1) * 384], ptb[:])

                # D-blur, img out [d', (hl, w)] via SBUF staging
                for m in range(2):
                    msl = slice(m * 96, (m + 1) * 96)
                    zi = zip_.tile([96, HB * W], F32, tag="zi")
                    for q in range(HB * W // 512):
                        sl = slice(q * 512, (q + 1) * 512)
                        psd = psp.tile([96, 512], F32, tag="ps")
                        nc.tensor.matmul(psd[:], gda[:, msl], zda[:, sl], start=True, stop=False)
                        nc.tensor.matmul(psd[:], gdb[:, msl], zdb[:, sl], start=False, stop=True)
                        nc.scalar.copy(zi[:, sl], psd[:])
                    nc.sync.dma_start(
                        bass.AP(img_h, m * 96 * HC * W + h0 * W,
                                [[HC * W, 96], [1, HB * W]]),
                        zi[:],
                    )
    nc.compile()
    return nc


def _host_prep(x, small_bias, sigma01, labels, source_values, dest_values):
    Wd = _lin_weights(SMALL, D)
    Whm = _lin_weights(SMALL, H)
    Wwm = _lin_weights(SMALL, W)
    eye_bf = np.eye(128, dtype=ml_dtypes.bfloat16)
    eye_f32 = np.eye(128, dtype=np.float32)

    mapping = np.zeros(TABLE, np.int32)
    mapping[np.asarray(source_values, np.int64)] = np.asarray(dest_values, np.int64).astype(np.int32)
    tabf = mapping[:N_LABELS].astype(np.float32)
    tab_rep = np.broadcast_to(tabf, (128, N_LABELS)).copy()

    in_maps = []
    for c in range(8):
        b, half = c // 2, c % 2
        h0 = half * HC
        hidx = np.clip(np.arange(h0 - P, h0 + HC + P), 0, H - 1)

        xs = np.ascontiguousarray(np.asarray(x[b, 0], np.float32)[:, hidx, :])

        sm = np.asarray(small_bias[b, 0], np.float64) * BIAS_STD
        Cydw = np.einsum("xyz,dx,wz->ydw", sm, Wd, Wwm).reshape(4, D * W)
        WhT = np.ascontiguousarray(Whm[hidx, :].T)

        g3 = _gauss_kernels(np.asarray(sigma01[b], np.float64) * MAX_SIGMA)
        Gh = _slab_toeplitz(g3[1])
        Gw = _edge_folded_toeplitz(g3[2], W)
        Gd = _edge_folded_toeplitz(g3[0], D)

        lab = np.asarray(labels[b, 0][:, h0:h0 + HC, :], np.int16).reshape(128, FLAB)

        in_maps.append({
            "xs": xs,
            "cydw": Cydw.astype(ml_dtypes.bfloat16),
            "wht": WhT.astype(ml_dtypes.bfloat16),
            "gh": Gh.astype(ml_dtypes.bfloat16),
            "gw": Gw.astype(ml_dtypes.bfloat16),
            "gd": Gd.astype(ml_dtypes.bfloat16),
            "lab": np.ascontiguousarray(lab),
            "tab": tab_rep,
            "idm": eye_bf,
            "idmf": eye_f32,
        })
    return in_maps


def kernel(x, small_bias, sigma01, labels, source_values, dest_values):
    if "nc" not in _CACHE:
        _CACHE["nc"] = _build_program()
    nc = _CACHE["nc"]

    in_maps = _host_prep(x, small_bias, sigma01, labels, source_values, dest_values)
    res = run_bass_kernel_spmd(nc, in_maps, core_ids=list(range(8)))

    img = np.empty((B, C, D, H, W), np.float32)
    labels_out = np.empty((B, C, D, H, W), np.int32)
    for c in range(8):
        b, half = c // 2, c % 2
        h0 = half * HC
        r = res.results[c]
        img[b, 0, :, h0:h0 + HC, :] = r["img"].reshape(D, HC, W)
        labels_out[b, 0, :, h0:h0 + HC, :] = (
            r["labo"].reshape(D, HC, W).astype(np.int32))
    return img, labels_out



# revision 2
# speedup vs baseline: 1.4799x; 1.4799x over previous
"""Trainium2 Bass kernel v2 for nn_BrainGeneratorModel.

8 cores = (sample b 0..3) x (H-half 0..1); per core: slab [D=192, HS=120, W=192]
-> img [192, 96, 192] f32 + label remap of [128, 27648] i16.

Image pipeline (bf16 matmuls, banded split => each blur one 128-contraction):
  A) per d-batch (DB=8): bias matmul (K=4) -> exp (ACT, bf16) -> x*exp (DVE)
     -> H-blur (K=120) -> PE transpose w onto partitions (rows 0:128 & 64:192)
     -> W-blur (banded halves) -> y2 DRAM bf16 [w', d, h'].
  B) load y2 rows whole; per hl-batch: PE transpose d onto partitions (strided
     moving) -> D-blur (banded halves) -> img f32.

Labels (bit-plane, chunks of FC): v = 2^l via f32 exponent-encode;
per output bit b: pb = ((v & mask_b) != 0) * 2^b; sequential i16 accumulate.
Masks arrive via a [128, 8] u32 input tile (exact 32-bit values).
"""

import sys

for _p in ("/opt/trn_rl_repo",):
    if _p not in sys.path:
        sys.path.insert(0, _p)

import numpy as np
import ml_dtypes

import concourse.bass as bass
import concourse.mybir as mybir
import concourse.bacc as bacc
import concourse.tile as tile
from concourse.bass_utils import run_bass_kernel_spmd

F32 = mybir.dt.float32
BF16 = mybir.dt.bfloat16
I16 = mybir.dt.int16
I32 = mybir.dt.int32
U32 = mybir.dt.uint32
ALU = mybir.AluOpType

B, C, D, H, W = 4, 1, 192, 192, 192
SMALL = 4
BIAS_STD = 0.7
MAX_SIGMA = 3.0
TRUNCATE = 4.0
K = 2 * int(TRUNCATE * MAX_SIGMA) + 1  # 25
P = K // 2  # 12
N_LABELS = 32
TABLE = 128

HC = 96            # interior H rows per core
HS = 120           # slab rows
DB = 16            # d-batch size (stage A)
NB_A = D // DB     # 12
FA = DB * W        # 3072
HB = 8             # hl-batch size (stage B)
NB_B = HC // HB    # 12
FLAB = D * HC * W // 128  # 27648
NLC = 18           # label chunks
FC = FLAB // NLC   # 1536

_CACHE = {}


def _lin_weights(n_in, n_out):
    pos = np.linspace(0.0, n_in - 1.0, n_out, dtype=np.float64)
    i0 = np.clip(np.floor(pos).astype(np.int64), 0, n_in - 2)
    f = pos - i0
    Wm = np.zeros((n_out, n_in), np.float64)
    r = np.arange(n_out)
    np.add.at(Wm, (r, i0), 1.0 - f)
    np.add.at(Wm, (r, i0 + 1), f)
    return Wm


def _gauss_kernels(sigma3):
    ar = np.arange(K, dtype=np.float64) - K // 2
    out = np.zeros((3, K), np.float64)
    for i, sg in enumerate(sigma3):
        s = max(float(sg), 1e-3)
        g = np.exp(-0.5 * ar * ar / (s * s))
        g = g / g.sum()
        if float(sg) >= 0.01:
            out[i] = g
        else:
            out[i, K // 2] = 1.0
    return out


def _edge_folded_toeplitz(g, n):
    M = np.zeros((n, n), np.float64)
    for j in range(n):
        for t in range(K):
            src = min(max(j + t - P, 0), n - 1)
            M[src, j] += g[t]
    return M


def _slab_toeplitz(g):
    M = np.zeros((HS, HC), np.float64)
    for j in range(HC):
        for t in range(K):
            M[j + t, j] += g[t]
    return M


def _imm_i32(m):
    """Signed-int representation of a u32 mask for exact f64 imm transport."""
    m = int(m) & 0xFFFFFFFF
    return float(m - (1 << 32) if m >= (1 << 31) else m)


def _emit_label_chunk(nc, ltp, labp, lab_h, labo_h, cc, masks, pending):
    """One label chunk [128, FC]: bit-plane remap, i16 out. DVE + 2 ACT passes.

    The output DMA of the PREVIOUS chunk is issued here (its data is long
    ready), so SP never head-of-line blocks on a DVE wait."""
    while pending:
        po, pacc = pending.pop(0)
        nc.sync.dma_start(po, pacc)
    sl = slice(cc * FC, (cc + 1) * FC)
    lt = ltp.tile([128, FC], I16, tag="lt")
    nc.sync.dma_start(lt[:], lab_h.ap()[:, sl])
    # enc = int((l + 127) * 2^23)  -- fused on ACT: Copy(l*2^23 + 127*2^23) -> i32
    enc = labp.tile([128, FC], I32, tag="enc")
    nc.scalar.activation(enc[:], lt[:], mybir.ActivationFunctionType.Copy,
                         bias=127.0 * float(1 << 23), scale=float(1 << 23))
    # vu = u32(f32-bitcast(enc)) = 2^l  (exact for all l in [0,32))
    vu = labp.tile([128, FC], U32, tag="vu")
    nc.scalar.copy(vu[:], enc[:].bitcast(F32))

    def mk_pb(b, tag):
        eb = labp.tile([128, FC], U32, tag="eb")
        nc.vector.tensor_scalar(eb[:], vu[:], _imm_i32(masks[b]), None,
                                ALU.bitwise_and)
        pb = labp.tile([128, FC], I16, tag=tag)
        nc.vector.tensor_scalar(pb[:], eb[:], 0.0, float(2 ** b),
                                ALU.not_equal, ALU.mult)
        return pb

    # fold everything on DVE: acc = sum_b pb_b
    acc = labp.tile([128, FC], I16, tag="acc")
    pb0 = mk_pb(0, "pbA")
    pb1 = mk_pb(1, "pbB")
    nc.vector.tensor_tensor(acc[:], pb0[:], pb1[:], ALU.add)
    for b in range(2, 7):
        pbx = mk_pb(b, "pbA" if b % 2 == 0 else "pbB")
        nc.vector.tensor_tensor(acc[:], acc[:], pbx[:], ALU.add)
    pending.append((labo_h.ap()[:, sl], acc[:]))


def _build_program(masks):
    nc = bacc.Bacc("TRN2", target_bir_lowering=False, debug=False)

    # ---- external inputs (per core) ----
    xs_h = nc.dram_tensor("xs", [D, HS, W], BF16, kind="ExternalInput")
    c_h = nc.dram_tensor("cydw", [4, D * W], BF16, kind="ExternalInput")
    wht_h = nc.dram_tensor("wht", [4, HS], BF16, kind="ExternalInput")
    gh_h = nc.dram_tensor("gh", [HS, HC], BF16, kind="ExternalInput")
    gwa_h = nc.dram_tensor("gwa", [128, 96], BF16, kind="ExternalInput")
    gwc_h = nc.dram_tensor("gwc", [128, 96], BF16, kind="ExternalInput")
    gda_h = nc.dram_tensor("gda", [128, 96], BF16, kind="ExternalInput")
    gdc_h = nc.dram_tensor("gdc", [128, 96], BF16, kind="ExternalInput")
    id_h = nc.dram_tensor("idm", [128, 128], BF16, kind="ExternalInput")
    lab_h = nc.dram_tensor("lab", [128, FLAB], I16, kind="ExternalInput")

    # ---- external outputs ----
    img_h = nc.dram_tensor("img", [D, HC, W], F32, kind="ExternalOutput")
    labo_h = nc.dram_tensor("labo", [128, FLAB], I16, kind="ExternalOutput")

    with tile.TileContext(nc) as tc:
        with (
            tc.tile_pool(name="consts", bufs=1) as cst,
            tc.tile_pool(name="sxp", bufs=2) as sxp,
            tc.tile_pool(name="cbp", bufs=2) as cbp,
            tc.tile_pool(name="ebp", bufs=3) as ebp,
            tc.tile_pool(name="xbp", bufs=2) as xbp,
            tc.tile_pool(name="xhp", bufs=2) as xhp,
            tc.tile_pool(name="zwp", bufs=2) as zwp,
            tc.tile_pool(name="zvp", bufs=2) as zvp,
            tc.tile_pool(name="ybp", bufs=1) as ybp,
            tc.tile_pool(name="zdp", bufs=2) as zdp,
            tc.tile_pool(name="zip", bufs=2) as zip_,
            tc.tile_pool(name="ltp", bufs=2) as ltp,
            tc.tile_pool(name="labp", bufs=1) as labp,
            tc.tile_pool(name="ps", bufs=8, space="PSUM") as psp,
            tc.tile_pool(name="dram", bufs=1, space="DRAM") as drp,
        ):
            # ---- constants ----
            ght = cst.tile([HS, HC], BF16)
            nc.sync.dma_start(ght[:], gh_h.ap())
            gwa = cst.tile([128, 96], BF16)
            nc.sync.dma_start(gwa[:], gwa_h.ap())
            gwc = cst.tile([128, 96], BF16)
            nc.sync.dma_start(gwc[:], gwc_h.ap())
            gda = cst.tile([128, 96], BF16)
            nc.sync.dma_start(gda[:], gda_h.ap())
            gdc = cst.tile([128, 96], BF16)
            nc.sync.dma_start(gdc[:], gdc_h.ap())
            whtt = cst.tile([4, HS], BF16)
            nc.sync.dma_start(whtt[:], wht_h.ap())
            idt = cst.tile([128, 128], BF16)
            nc.sync.dma_start(idt[:], id_h.ap())

            # y2 scratch: [w', d, h'] bf16
            y2 = drp.tile([192, D * HC], BF16)

            lab_done = 0
            pending = []

            # ================= stage A =================
            for ib in range(NB_A):
                d0 = ib * DB
                sx = sxp.tile([HS, FA], BF16, tag="sx")
                nc.sync.dma_start(
                    sx[:],
                    bass.AP(xs_h, d0 * HS * W, [[W, HS], [HS * W, DB], [1, W]]),
                )
                cb = cbp.tile([4, FA], BF16, tag="cb")
                nc.sync.dma_start(cb[:], c_h.ap()[:, d0 * W:(d0 + DB) * W])

                xb = xbp.tile([HS, FA], BF16, tag="xb")
                xh = xhp.tile([HC, FA], BF16, tag="xh")
                for q in range(FA // 512):
                    sl = slice(q * 512, (q + 1) * 512)
                    psb = psp.tile([HS, 512], F32, tag="ps")
                    nc.tensor.matmul(psb[:], whtt[:], cb[:, sl], start=True, stop=True)
                    eb = ebp.tile([HS, 512], BF16, tag="eb")
                    nc.scalar.activation(eb[:], psb[:], mybir.ActivationFunctionType.Exp)
                    nc.gpsimd.tensor_tensor(xb[:, sl], sx[:, sl], eb[:], ALU.mult)
                    psh = psp.tile([HC, 512], F32, tag="ps")
                    nc.tensor.matmul(psh[:], ght[:], xb[:, sl], start=True, stop=True)
                    nc.scalar.copy(xh[:, sl], psh[:])

                # T1: w onto partitions; zwA rows 0..127, zwC rows 64..191
                # layout [128, (dl, h')]
                zwA = zwp.tile([128, DB * HC], BF16, tag="zwA")
                zwC = zwp.tile([128, DB * HC], BF16, tag="zwC")
                for g in range(DB // 4):
                    ptA = psp.tile([128, 4 * HC], BF16, tag="ps")
                    ptC = psp.tile([128, 4 * HC], BF16, tag="ps")
                    for t in range(4):
                        dl = g * 4 + t
                        nc.tensor.transpose(
                            ptA[:, t * HC:(t + 1) * HC],
                            xh[:, dl * W: dl * W + 128], idt[0:HC, 0:HC])
                        nc.tensor.transpose(
                            ptC[:, t * HC:(t + 1) * HC],
                            xh[:, dl * W + 64: dl * W + 192], idt[0:HC, 0:HC])
                    nc.scalar.copy(zwA[:, g * 4 * HC:(g + 1) * 4 * HC], ptA[:])
                    nc.scalar.copy(zwC[:, g * 4 * HC:(g + 1) * 4 * HC], ptC[:])

                # W-blur: m=0 from zwA (w rows 0..127), m=1 from zwC (64..191)
                for m, (gw_, zw_) in enumerate(((gwa, zwA), (gwc, zwC))):
                    zv = zvp.tile([96, DB * HC], BF16, tag="zv")
                    for q in range(DB * HC // 512):
                        sl = slice(q * 512, (q + 1) * 512)
                        psw = psp.tile([96, 512], F32, tag="ps")
                        nc.tensor.matmul(psw[:], gw_[:], zw_[:, sl], start=True, stop=True)
                        nc.scalar.copy(zv[:, sl], psw[:])
                    # zv free = (dl, h') == y2 [d0*HC : (d0+DB)*HC] contiguous
                    nc.gpsimd.dma_start(
                        bass.AP(y2.tensor,
                                y2[:].offset + m * 96 * D * HC + d0 * HC,
                                [[D * HC, 96], [1, DB * HC]]),
                        zv[:],
                    )

                if ib >= 2 and lab_done < NLC // 2:
                    _emit_label_chunk(nc, ltp, labp, lab_h, labo_h,
                                      lab_done, masks, pending)
                    lab_done += 1

            # ================= stage B =================
            ybA = ybp.tile([96, D * HC], BF16)
            nc.sync.dma_start(ybA[:], bass.AP(y2.tensor, y2[:].offset,
                                              [[D * HC, 96], [1, D * HC]]))
            ybC = ybp.tile([96, D * HC], BF16)
            nc.sync.dma_start(ybC[:], bass.AP(y2.tensor, y2[:].offset + 96 * D * HC,
                                              [[D * HC, 96], [1, D * HC]]))

            for jb in range(NB_B):
                h0 = jb * HB
                # T2: d onto partitions; strided moving reads from yb tiles
                # zdA rows d 0..127, zdC rows d 64..191; layout [*, (hl, w)]
                zdA = zdp.tile([128, HB * W], BF16, tag="zdA")
                zdC = zdp.tile([128, HB * W], BF16, tag="zdC")
                for g in range(HB // 4):
                    ptA = psp.tile([128, 768], BF16, tag="ps")
                    ptC = psp.tile([128, 768], BF16, tag="ps")
                    for t in range(4):
                        hl = g * 4 + t
                        colA = bass.AP(ybA.tensor, ybA[:].offset + (h0 + hl),
                                       [ybA[:].ap[0], [HC, 128]])
                        colB = bass.AP(ybC.tensor, ybC[:].offset + (h0 + hl),
                                       [ybC[:].ap[0], [HC, 128]])
                        nc.tensor.transpose(ptA[:, t * 192 + 0:t * 192 + 96],
                                            colA, idt[0:96, 0:96])
                        nc.tensor.transpose(ptA[:, t * 192 + 96:t * 192 + 192],
                                            colB, idt[0:96, 0:96])
                        colA2 = bass.AP(ybA.tensor, ybA[:].offset + (h0 + hl) + 64 * HC,
                                        [ybA[:].ap[0], [HC, 128]])
                        colB2 = bass.AP(ybC.tensor, ybC[:].offset + (h0 + hl) + 64 * HC,
                                        [ybC[:].ap[0], [HC, 128]])
                        nc.tensor.transpose(ptC[:, t * 192 + 0:t * 192 + 96],
                                            colA2, idt[0:96, 0:96])
                        nc.tensor.transpose(ptC[:, t * 192 + 96:t * 192 + 192],
                                            colB2, idt[0:96, 0:96])
                    nc.scalar.copy(zdA[:, g * 768:(g + 1) * 768], ptA[:])
                    nc.scalar.copy(zdC[:, g * 768:(g + 1) * 768], ptC[:])

                # D-blur: n=0 -> img rows 0..95 from zdA; n=1 -> 96..191 from zdC
                for n, (gd_, zd_) in enumerate(((gda, zdA), (gdc, zdC))):
                    zi = zip_.tile([96, HB * W], F32, tag="zi")
                    for q in range(HB * W // 512):
                        sl = slice(q * 512, (q + 1) * 512)
                        psd = psp.tile([96, 512], F32, tag="ps")
                        nc.tensor.matmul(psd[:], gd_[:], zd_[:, sl], start=True, stop=True)
                        nc.scalar.copy(zi[:, sl], psd[:])
                    nc.scalar.dma_start(
                        bass.AP(img_h, n * 96 * HC * W + h0 * W,
                                [[HC * W, 96], [1, HB * W]]),
                        zi[:],
                    )

                if lab_done < NLC - 2:
                    _emit_label_chunk(nc, ltp, labp, lab_h, labo_h,
                                      lab_done, masks, pending)
                    lab_done += 1

            while lab_done < NLC:
                _emit_label_chunk(nc, ltp, labp, lab_h, labo_h,
                                  lab_done, masks, pending)
                lab_done += 1
            while pending:
                po, pacc = pending.pop(0)
                nc.sync.dma_start(po, pacc)

    nc.compile()
    return nc


def _host_prep(x, small_bias, sigma01, labels, source_values, dest_values):
    Wd = _lin_weights(SMALL, D)
    Whm = _lin_weights(SMALL, H)
    Wwm = _lin_weights(SMALL, W)
    eye_bf = np.eye(128, dtype=ml_dtypes.bfloat16)

    mapping = np.zeros(TABLE, np.int64)
    mapping[np.asarray(source_values, np.int64)] = np.asarray(dest_values, np.int64)

    x_np = np.asarray(x, np.float32)
    lab_np = np.asarray(labels)

    in_maps = []
    for c in range(8):
        b, half = c // 2, c % 2
        h0 = half * HC
        hidx = np.clip(np.arange(h0 - P, h0 + HC + P), 0, H - 1)

        xs = np.ascontiguousarray(x_np[b, 0][:, hidx, :]).astype(ml_dtypes.bfloat16)

        sm = np.asarray(small_bias[b, 0], np.float64) * BIAS_STD
        Cydw = np.einsum("xyz,dx,wz->ydw", sm, Wd, Wwm).reshape(4, D * W)
        WhT = np.ascontiguousarray(Whm[hidx, :].T)

        g3 = _gauss_kernels(np.asarray(sigma01[b], np.float64) * MAX_SIGMA)
        Gh = _slab_toeplitz(g3[1])
        Gw = _edge_folded_toeplitz(g3[2], W)
        Gd = _edge_folded_toeplitz(g3[0], D)

        lab = lab_np[b, 0][:, h0:h0 + HC, :].astype(np.int16).reshape(128, FLAB)

        in_maps.append({
            "xs": xs,
            "cydw": Cydw.astype(ml_dtypes.bfloat16),
            "wht": WhT.astype(ml_dtypes.bfloat16),
            "gh": Gh.astype(ml_dtypes.bfloat16),
            "gwa": np.ascontiguousarray(Gw[0:128, 0:96]).astype(ml_dtypes.bfloat16),
            "gwc": np.ascontiguousarray(Gw[64:192, 96:192]).astype(ml_dtypes.bfloat16),
            "gda": np.ascontiguousarray(Gd[0:128, 0:96]).astype(ml_dtypes.bfloat16),
            "gdc": np.ascontiguousarray(Gd[64:192, 96:192]).astype(ml_dtypes.bfloat16),
            "idm": eye_bf,
            "lab": np.ascontiguousarray(lab),
        })
    return in_maps


def _label_masks(source_values, dest_values):
    mapping = np.zeros(TABLE, np.int64)
    mapping[np.asarray(source_values, np.int64)] = np.asarray(dest_values, np.int64)
    masks = []
    for b in range(7):
        m = 0
        for l in range(N_LABELS):
            if (int(mapping[l]) >> b) & 1:
                m |= 1 << l
        masks.append(m)
    return tuple(masks)


def kernel(x, small_bias, sigma01, labels, source_values, dest_values):
    masks = _label_masks(source_values, dest_values)
    if _CACHE.get("masks") != masks:
        _CACHE["nc"] = _build_program(masks)
        _CACHE["masks"] = masks
    nc = _CACHE["nc"]

    in_maps = _host_prep(x, small_bias, sigma01, labels, source_values, dest_values)
    res = run_bass_kernel_spmd(nc, in_maps, core_ids=list(range(8)))

    img = np.empty((B, C, D, H, W), np.float32)
    labels_out = np.empty((B, C, D, H, W), np.int32)
    for c in range(8):
        b, half = c // 2, c % 2
        h0 = half * HC
        r = res.results[c]
        img[b, 0, :, h0:h0 + HC, :] = r["img"].reshape(D, HC, W)
        labels_out[b, 0, :, h0:h0 + HC, :] = (
            r["labo"].reshape(D, HC, W).astype(np.int32))
    return img, labels_out
